# revision 3
# baseline (speedup 1.0000x reference)
"""GCN layer kernel for Trainium2 (Bass/Tile), data-parallel over batch.

Per core (one batch element):
    out = relu(D^-1/2 A D^-1/2 (X W^T + b))

Host-side prep per core (numpy: dtype/layout marshaling + the O(N^2) row-sum):
  - A^T cast to bf16. The tensor engine contracts over partitions, so A's
    contraction index (its column) must live on partitions; shipping A^T makes
    every matmul stationary chunk a contiguous 128-col slice and halves HBM
    traffic vs f32 (the 16 MB f32 A load was the old bottleneck).
  - X^T, W^T, b cast to bf16 (mm1 stationary wants features on partitions).
  - d = deg^-1/2 shipped as a [128, 16] f32 column table (deg = A row sums).
    deg needs full A rows, which live across all 16 A^T tiles on device; doing
    it on host removes a global barrier that would serialize loads vs matmul.

Device schedule (per core), paced by the 16 x 1 MB A^T tile DMA stream:
  - HWDGE loads on the SP ring: d/b/X^T/W^T first, then the A^T tiles.
  - ~3us of junk 1-partition matmuls during the DMA head so the PE HAM
    clock-gate is warm (2.4 GHz) when real work starts.
  - per round k: mm1_k = bias (ones-outer-product) + X^T_k @ W^T chunks in one
    rotating PSUM bank; ACT drains y_k = d_k * psum to bf16; then 14 products
    acc[mu] += AT(k, mu).T @ y_k for mu = 2..15, accumulated over k in PSUM.
    The 14 accumulators stay live in 7 banks, 2 per bank: one start=True
    clears the bank's has_written bits, the second half-group's first write
    lands on cleared bits (overwrite+set), later writes accumulate per
    element. PE stays ~70% busy inside the DMA stream, no program-order stall.
  - tail: mu = 0, 1 products for all k reuse the mm1 bank after its last
    drain; they overlap the output drains/DMAs of mu 2..15.
  - drains relu(d_mu * acc[mu]) go to one staging tile; output leaves as 4 x
    512 KB DMAs on the (by then idle) SP ring, tail quarter last.
"""

from contextlib import ExitStack

import ml_dtypes
import numpy as np

import concourse.bacc as bacc
import concourse.mybir as mybir
import concourse.tile as tile
from concourse.bass_utils import run_bass_kernel_spmd

B = 8
N = 2048
F = 256
P = 128
NT = N // P  # 16 row tiles
FT = F // P  # 2 feature tiles
F32 = mybir.dt.float32
BF16 = mybir.dt.bfloat16
COPY = mybir.ActivationFunctionType.Copy
RELU = mybir.ActivationFunctionType.Relu
BF = ml_dtypes.bfloat16

TAIL = 2  # mu tiles computed after the streaming rounds (share the mm1 bank)
WARMUP_MMS = 28


def _emit(ctx: ExitStack, tc: tile.TileContext, AT, XT, WT, BIAS, DCOL, OUT):
    nc = tc.nc

    const = ctx.enter_context(tc.tile_pool(name="const", bufs=1))
    atp = ctx.enter_context(tc.tile_pool(name="atp", bufs=1))
    psum = ctx.enter_context(tc.tile_pool(name="psum", bufs=7, space="PSUM"))

    xt_sb = const.tile([P, FT * N], BF16, tag="xt")
    wt_sb = const.tile([P, FT * F], BF16, tag="wt")
    dcol = const.tile([P, NT], F32, tag="dcol")
    bias_sb = const.tile([1, F], BF16, tag="bias")
    ones1 = const.tile([1, P], BF16, tag="ones")
    y_big = const.tile([P, NT * F], BF16, tag="y")
    out_big = const.tile([P, NT * F], F32, tag="out")
    at_big = atp.tile([P, NT * N], BF16, tag="at")

    # input DMAs (HWDGE, SP ring): mm1 operands first, then A^T row tiles
    nc.sync.dma_start(out=dcol[:, :], in_=DCOL[:, :])
    nc.sync.dma_start(out=bias_sb[:, :], in_=BIAS[:, :])
    for phi in range(FT):
        nc.sync.dma_start(
            out=xt_sb[:, phi * N : (phi + 1) * N], in_=XT[phi * P : (phi + 1) * P, :]
        )
        nc.sync.dma_start(
            out=wt_sb[:, phi * F : (phi + 1) * F], in_=WT[phi * P : (phi + 1) * P, :]
        )
    for k in range(NT):
        nc.sync.dma_start(
            out=at_big[:, k * N : (k + 1) * N], in_=AT[k * P : (k + 1) * P, :]
        )

    nc.vector.memset(ones1[:, :], 1.0)

    # ---- PE warmup: keep HAM busy during the DMA head (junk results) ----
    junk = psum.tile([P, 2 * F], F32, tag="mm1", bufs=1, name="junk")
    for _ in range(WARMUP_MMS):
        nc.tensor.matmul(junk[:, :P], ones1[:, :], ones1[:, :], start=True, stop=True)

    accs = [
        psum.tile([P, 2 * F], F32, tag="bank", name=f"acc_{bk}")
        for bk in range((NT - TAIL) // 2)
    ]

    def emit_product(k, mu, acc_ap, h):
        nc.tensor.matmul(
            acc_ap[:, h * F : (h + 1) * F],
            at_big[:, k * N + mu * P : k * N + (mu + 1) * P],
            y_big[:, k * F : (k + 1) * F],
            # one bank clear per bank: h==1's first write lands on cleared
            # has_written bits (overwrite+set), so no second start
            start=(k == 0 and h == 0),
            stop=(k == NT - 1),
            skip_group_check=True,
        )

    def emit_drain(mu, acc_ap, h):
        nc.scalar.activation(
            out_big[:, mu * F : (mu + 1) * F],
            acc_ap[:, h * F : (h + 1) * F],
            RELU,
            scale=dcol[:, mu : mu + 1],
        )

    def emit_out_quarter(q):
        nc.sync.dma_start(
            out=OUT[4 * P * q : 4 * P * (q + 1), :].rearrange(
                "(m p) f -> p m f", p=P
            ),
            in_=out_big[:, 4 * F * q : 4 * F * (q + 1)].rearrange(
                "p (m f) -> p m f", f=F
            ),
        )

    # ---- streaming rounds: mm1_k + y_k + products (k, mu=TAIL..15) ----
    for k in range(NT):
        tp = psum.tile([P, 2 * F], F32, tag="mm1", bufs=1, name=f"mm1_{k}")
        nc.tensor.matmul(tp[:, :F], ones1[:, :], bias_sb[:, :], start=True, stop=False)
        for phi in range(FT):
            nc.tensor.matmul(
                tp[:, :F],
                xt_sb[:, phi * N + k * P : phi * N + (k + 1) * P],
                wt_sb[:, phi * F : (phi + 1) * F],
                start=False,
                stop=(phi == FT - 1),
            )
        nc.scalar.activation(
            y_big[:, k * F : (k + 1) * F], tp[:, :F], COPY, scale=dcol[:, k : k + 1]
        )
        for mu in range(TAIL, NT):
            bk, h = divmod(mu - TAIL, 2)
            emit_product(k, mu, accs[bk], h)

    # ---- drains for streamed mu, then output quarters 1..3 ----
    for mu in range(TAIL, NT):
        bk, h = divmod(mu - TAIL, 2)
        emit_drain(mu, accs[bk], h)
    for q in range(1, 4):
        emit_out_quarter(q)

    # ---- tail: mu = 0..TAIL-1 reuse the mm1 bank, then quarter 0 ----
    acc_tail = psum.tile([P, 2 * F], F32, tag="mm1", bufs=1, name="acc_tail")
    for k in range(NT):
        for mu in range(TAIL):
            emit_product(k, mu, acc_tail, mu)
    for mu in range(TAIL):
        emit_drain(mu, acc_tail, mu)
    emit_out_quarter(0)


_cached_nc = None


def _build():
    nc = bacc.Bacc("TRN2", target_bir_lowering=False, debug=False)
    AT = nc.dram_tensor("at", [N, N], BF16, kind="ExternalInput").ap()
    XT = nc.dram_tensor("xt", [F, N], BF16, kind="ExternalInput").ap()
    WT = nc.dram_tensor("wt", [F, F], BF16, kind="ExternalInput").ap()
    BIAS = nc.dram_tensor("bias", [1, F], BF16, kind="ExternalInput").ap()
    DCOL = nc.dram_tensor("dcol", [P, NT], F32, kind="ExternalInput").ap()
    OUT = nc.dram_tensor("out", [N, F], F32, kind="ExternalOutput").ap()
    with tile.TileContext(nc) as tc:
        with ExitStack() as ctx:
            _emit(ctx, tc, AT, XT, WT, BIAS, DCOL, OUT)
    nc.compile()
    return nc


def get_nc():
    global _cached_nc
    if _cached_nc is None:
        _cached_nc = _build()
    return _cached_nc


def make_in_maps(node_features, adj_matrix, W, b):
    node_features = np.asarray(node_features, dtype=np.float32)
    adj_matrix = np.asarray(adj_matrix, dtype=np.float32)
    wt = np.ascontiguousarray(np.asarray(W, dtype=np.float32).T.astype(BF))
    bias = np.ascontiguousarray(
        np.asarray(b, dtype=np.float32).astype(BF).reshape(1, F)
    )
    maps = []
    for c in range(B):
        adj = adj_matrix[c]
        deg = adj.sum(axis=1, dtype=np.float32)
        with np.errstate(divide="ignore"):
            d = deg**-0.5
        d = np.where(np.isfinite(d), d, 0.0).astype(np.float32)
        maps.append(
            {
                "at": np.ascontiguousarray(adj.astype(BF).T),
                "xt": np.ascontiguousarray(node_features[c].T.astype(BF)),
                "wt": wt,
                "bias": bias,
                "dcol": np.ascontiguousarray(d.reshape(NT, P).T),
            }
        )
    return maps


def kernel(node_features, adj_matrix, W, b):
    nc = get_nc()
    in_maps = make_in_maps(node_features, adj_matrix, W, b)
    res = run_bass_kernel_spmd(nc, in_maps, core_ids=list(range(B)))
    return np.stack([r["out"] for r in res.results], axis=0)


# revision 4
# speedup vs baseline: 1.2074x; 1.2074x over previous
"""GCN layer kernel for Trainium2 (Bass/Tile), data-parallel over batch.

Per core (one batch element):
    out = relu(D^-1/2 A D^-1/2 (X W^T + b))

Host-side prep per core (numpy: dtype/layout marshaling + the O(N^2) row-sum):
  - A^T cast to bf16. The tensor engine contracts over partitions, so A's
    contraction index (its column) must live on partitions; shipping A^T makes
    every matmul stationary chunk a contiguous 128-col slice and halves HBM
    traffic vs f32 (the 16 MB f32 A load was the old bottleneck).
  - X^T, W^T, b cast to bf16 (mm1 stationary wants features on partitions).
  - d = deg^-1/2 shipped as a [128, 16] f32 column table (deg = A row sums).
    deg needs full A rows, which live across all 16 A^T tiles on device; doing
    it on host removes a global barrier that would serialize loads vs matmul.
  - Output returns as bf16 (halves the writeback), upcast to f32 on host.

Device schedule (per core), paced by the 16 x 1 MB A^T tile DMA stream:
  - HWDGE loads on the SP ring: d/b/X^T/W^T first, then the A^T tiles.
  - All 16 y_k = d_k * (X_k W^T + b) land first: mm1_k double-buffers through
    the two halves of ONE PSUM bank (start=True only clears has_written bits,
    not data, so the other half's pending ACT drain is unaffected); PE never
    waits for the ACT drains. Doubles as the PE HAM warmup.
  - main matmul streams per arriving A^T tile k: 16 products
    acc[mu] += AT(k, mu).T @ y_k accumulated over k in PSUM. mu 0..13 pack 2
    per bank in 7 banks (one start=True bank clear; the second half-group's
    first write lands on cleared has_written bits and overwrites), mu 14..15
    reuse the mm1 bank once y_15 has drained. PE rounds (~1.8us) run inside
    the DMA pace (~2.9us/tile), so the stream stays DMA-bound.
  - last round interleaves products, relu(d_mu * acc) drains (alternating
    ACT / DVE), and 4 x 256 KB output DMAs on the by-then-idle SP ring.
"""

from contextlib import ExitStack

import ml_dtypes
import numpy as np

import concourse.bacc as bacc
import concourse.mybir as mybir
import concourse.tile as tile
from concourse.bass_utils import run_bass_kernel_spmd

B = 8
N = 2048
F = 256
P = 128
NT = N // P  # 16 row tiles
FT = F // P  # 2 feature tiles
F32 = mybir.dt.float32
BF16 = mybir.dt.bfloat16
COPY = mybir.ActivationFunctionType.Copy
RELU = mybir.ActivationFunctionType.Relu
MULT = mybir.AluOpType.mult
MAX = mybir.AluOpType.max
BF = ml_dtypes.bfloat16


def _emit(ctx: ExitStack, tc: tile.TileContext, AT, XT, WT, BIAS, DCOL, OUT):
    nc = tc.nc

    const = ctx.enter_context(tc.tile_pool(name="const", bufs=1))
    atp = ctx.enter_context(tc.tile_pool(name="atp", bufs=1))
    psum = ctx.enter_context(tc.tile_pool(name="psum", bufs=7, space="PSUM"))

    xt_sb = const.tile([P, FT * N], BF16, tag="xt")
    wt_sb = const.tile([P, FT * F], BF16, tag="wt")
    dcol = const.tile([P, NT], F32, tag="dcol")
    bias_sb = const.tile([1, F], BF16, tag="bias")
    ones1 = const.tile([1, P], BF16, tag="ones")
    y_big = const.tile([P, NT * F], BF16, tag="y")
    out_big = const.tile([P, NT * F], BF16, tag="out")
    at_big = atp.tile([P, NT * N], BF16, tag="at")

    # input DMAs (HWDGE, SP ring): mm1 operands first, then A^T row tiles
    nc.sync.dma_start(out=dcol[:, :], in_=DCOL[:, :])
    nc.sync.dma_start(out=bias_sb[:, :], in_=BIAS[:, :])
    for phi in range(FT):
        nc.sync.dma_start(
            out=xt_sb[:, phi * N : (phi + 1) * N], in_=XT[phi * P : (phi + 1) * P, :]
        )
        nc.sync.dma_start(
            out=wt_sb[:, phi * F : (phi + 1) * F], in_=WT[phi * P : (phi + 1) * P, :]
        )
    for k in range(NT):
        nc.sync.dma_start(
            out=at_big[:, k * N : (k + 1) * N], in_=AT[k * P : (k + 1) * P, :]
        )

    nc.vector.memset(ones1[:, :], 1.0)

    # ---- all 16 y_k first, double-buffered through one PSUM bank's halves ----
    mm1buf = psum.tile([P, 2 * F], F32, tag="mm1", bufs=1, name="mm1buf")
    for k in range(NT):
        h = k % 2
        reg = mm1buf[:, h * F : (h + 1) * F]
        nc.tensor.matmul(
            reg, ones1[:, :], bias_sb[:, :], start=True, stop=False,
            skip_group_check=True,
        )
        for phi in range(FT):
            nc.tensor.matmul(
                reg,
                xt_sb[:, phi * N + k * P : phi * N + (k + 1) * P],
                wt_sb[:, phi * F : (phi + 1) * F],
                start=False,
                stop=(phi == FT - 1),
                skip_group_check=True,
            )
        nc.scalar.activation(
            y_big[:, k * F : (k + 1) * F], reg, COPY, scale=dcol[:, k : k + 1]
        )

    # ---- main matmul: acc[mu] = sum_k AT(k, mu).T @ y_k ----
    accs = [psum.tile([P, 2 * F], F32, tag="bank", name=f"acc_{bk}") for bk in range(7)]
    acc7 = psum.tile([P, 2 * F], F32, tag="mm1", bufs=1, name="acc7")

    def acc_of(mu):
        return (accs[mu // 2], mu % 2) if mu < 14 else (acc7, mu - 14)

    def emit_product(k, mu):
        acc_ap, h = acc_of(mu)
        nc.tensor.matmul(
            acc_ap[:, h * F : (h + 1) * F],
            at_big[:, k * N + mu * P : k * N + (mu + 1) * P],
            y_big[:, k * F : (k + 1) * F],
            # one bank clear per bank: h==1's first write lands on cleared
            # has_written bits (overwrite+set), so no second start
            start=(k == 0 and h == 0),
            stop=(k == NT - 1),
            skip_group_check=True,
        )

    def emit_drain(mu):
        acc_ap, h = acc_of(mu)
        src = acc_ap[:, h * F : (h + 1) * F]
        dst = out_big[:, mu * F : (mu + 1) * F]
        if mu % 2 == 0:
            nc.scalar.activation(dst, src, RELU, scale=dcol[:, mu : mu + 1])
        else:
            nc.vector.tensor_scalar(
                out=dst, in0=src, scalar1=dcol[:, mu : mu + 1], scalar2=0.0,
                op0=MULT, op1=MAX,
            )

    for k in range(NT - 1):
        for mu in range(NT):
            emit_product(k, mu)
    # last round: interleave products, drains, and output quarters
    for mu in range(NT):
        emit_product(NT - 1, mu)
        emit_drain(mu)
        if mu % 4 == 3:
            q = mu // 4
            nc.sync.dma_start(
                out=OUT[4 * P * q : 4 * P * (q + 1), :].rearrange(
                    "(m p) f -> p m f", p=P
                ),
                in_=out_big[:, 4 * F * q : 4 * F * (q + 1)].rearrange(
                    "p (m f) -> p m f", f=F
                ),
            )


_cached_nc = None


def _build():
    nc = bacc.Bacc("TRN2", target_bir_lowering=False, debug=False)
    AT = nc.dram_tensor("at", [N, N], BF16, kind="ExternalInput").ap()
    XT = nc.dram_tensor("xt", [F, N], BF16, kind="ExternalInput").ap()
    WT = nc.dram_tensor("wt", [F, F], BF16, kind="ExternalInput").ap()
    BIAS = nc.dram_tensor("bias", [1, F], BF16, kind="ExternalInput").ap()
    DCOL = nc.dram_tensor("dcol", [P, NT], F32, kind="ExternalInput").ap()
    OUT = nc.dram_tensor("out", [N, F], BF16, kind="ExternalOutput").ap()
    with tile.TileContext(nc) as tc:
        with ExitStack() as ctx:
            _emit(ctx, tc, AT, XT, WT, BIAS, DCOL, OUT)
    nc.compile()
    return nc


def get_nc():
    global _cached_nc
    if _cached_nc is None:
        _cached_nc = _build()
    return _cached_nc


def make_in_maps(node_features, adj_matrix, W, b):
    node_features = np.asarray(node_features, dtype=np.float32)
    adj_matrix = np.asarray(adj_matrix, dtype=np.float32)
    wt = np.ascontiguousarray(np.asarray(W, dtype=np.float32).T.astype(BF))
    bias = np.ascontiguousarray(
        np.asarray(b, dtype=np.float32).astype(BF).reshape(1, F)
    )
    maps = []
    for c in range(B):
        adj = adj_matrix[c]
        deg = adj.sum(axis=1, dtype=np.float32)
        with np.errstate(divide="ignore"):
            d = deg**-0.5
        d = np.where(np.isfinite(d), d, 0.0).astype(np.float32)
        maps.append(
            {
                "at": np.ascontiguousarray(adj.astype(BF).T),
                "xt": np.ascontiguousarray(node_features[c].T.astype(BF)),
                "wt": wt,
                "bias": bias,
                "dcol": np.ascontiguousarray(d.reshape(NT, P).T),
            }
        )
    return maps


def kernel(node_features, adj_matrix, W, b):
    nc = get_nc()
    in_maps = make_in_maps(node_features, adj_matrix, W, b)
    res = run_bass_kernel_spmd(nc, in_maps, core_ids=list(range(B)))
    return np.stack(
        [r["out"].astype(np.float32) for r in res.results], axis=0
    )


# revision 6
# speedup vs baseline: 1.2672x; 1.0496x over previous
"""GCN layer kernel for Trainium2 (Bass/Tile), data-parallel over batch.

Per core (one batch element):
    out = relu(D^-1/2 A D^-1/2 (X W^T + b))

Host-side prep per core (numpy: dtype/layout marshaling + the O(N^2) deg fold):
  - ATd = (D^-1/2 A)^T cast to bf16: A transposed (the tensor engine contracts
    over partitions, so A's contraction index must live on partitions), with
    the output-row scale D^-1/2 folded in so the PSUM drain is a pure relu.
    bf16 halves HBM traffic vs the f32 A load that bottlenecked the baseline.
  - X^T, W^T, b cast to bf16; d = deg^-1/2 as a [128, 16] f32 column table for
    the y = d * (XW^T + b) scale (deg needs full A rows, which live across all
    16 device tiles; host computes it to avoid a load/matmul barrier).
  - Output returns transposed [256, 2048] bf16; host casts + transposes back.

Device schedule (per core), paced by the 16 x 1 MB A^T tile DMA stream:
  - HWDGE loads on the SP ring: d/b/X^T/W^T first, then the A^T tiles.
  - All 16 y_k land first: mm1_k double-buffers through the two halves of ONE
    PSUM bank (start=True only clears has_written bits, not data, so the
    other half's pending ACT drain is unaffected). Doubles as PE HAM warmup.
  - main matmul, transposed-output form: out^T[o, r] = sum_c y[c, o] ATd[c, r]
    with y chunks stationary and ATd the moving operand in 512-wide slices:
    per tile k just 8 matmuls of N=512 into the 8 PSUM banks (o-chunk x
    r-quarter), LDWEIGHTS fully hidden under the 512-col streams, one clean
    accumulation group per bank. ~1.9us/round vs the ~2.5us DMA pace.
  - tail: relu drains (alternating ACT / DVE) into a bf16 staging tile, two
    512 KB output DMAs on the by-then-idle SP ring.
"""

from contextlib import ExitStack

import ml_dtypes
import numpy as np

import concourse.bacc as bacc
import concourse.mybir as mybir
import concourse.tile as tile
from concourse.bass_utils import run_bass_kernel_spmd

B = 8
N = 2048
F = 256
P = 128
NT = N // P  # 16 A^T row tiles
FT = F // P  # 2 feature tiles
RQ = 4  # 512-wide r-quarters per A^T tile
F32 = mybir.dt.float32
BF16 = mybir.dt.bfloat16
COPY = mybir.ActivationFunctionType.Copy
RELU = mybir.ActivationFunctionType.Relu
MAX = mybir.AluOpType.max
BF = ml_dtypes.bfloat16


def _emit(ctx: ExitStack, tc: tile.TileContext, AT, XT, WT, BIAS, DCOL, OUT):
    nc = tc.nc

    const = ctx.enter_context(tc.tile_pool(name="const", bufs=1))
    atp = ctx.enter_context(tc.tile_pool(name="atp", bufs=1))
    psum = ctx.enter_context(tc.tile_pool(name="psum", bufs=7, space="PSUM"))

    xt_sb = const.tile([P, FT * N], BF16, tag="xt")
    wt_sb = const.tile([P, FT * F], BF16, tag="wt")
    dcol = const.tile([P, NT], F32, tag="dcol")
    bias_sb = const.tile([1, F], BF16, tag="bias")
    ones1 = const.tile([1, P], BF16, tag="ones")
    y_big = const.tile([P, NT * F], BF16, tag="y")
    out_t = const.tile([P, FT * N], BF16, tag="out")
    at_big = atp.tile([P, NT * N], BF16, tag="at")

    # input DMAs (HWDGE, SP ring): mm1 operands first, then A^T row tiles
    nc.sync.dma_start(out=dcol[:, :], in_=DCOL[:, :])
    nc.sync.dma_start(out=bias_sb[:, :], in_=BIAS[:, :])
    for phi in range(FT):
        nc.sync.dma_start(
            out=xt_sb[:, phi * N : (phi + 1) * N], in_=XT[phi * P : (phi + 1) * P, :]
        )
        nc.sync.dma_start(
            out=wt_sb[:, phi * F : (phi + 1) * F], in_=WT[phi * P : (phi + 1) * P, :]
        )
    for k in range(NT):
        nc.sync.dma_start(
            out=at_big[:, k * N : (k + 1) * N], in_=AT[k * P : (k + 1) * P, :]
        )

    nc.vector.memset(ones1[:, :], 1.0)

    # ---- all 16 y_k first, double-buffered through one PSUM bank's halves ----
    mm1buf = psum.tile([P, 2 * F], F32, tag="mm1", bufs=1, name="mm1buf")
    for k in range(NT):
        h = k % 2
        reg = mm1buf[:, h * F : (h + 1) * F]
        nc.tensor.matmul(
            reg, ones1[:, :], bias_sb[:, :], start=True, stop=False,
            skip_group_check=True,
        )
        for phi in range(FT):
            nc.tensor.matmul(
                reg,
                xt_sb[:, phi * N + k * P : phi * N + (k + 1) * P],
                wt_sb[:, phi * F : (phi + 1) * F],
                start=False,
                stop=(phi == FT - 1),
                skip_group_check=True,
            )
        nc.scalar.activation(
            y_big[:, k * F : (k + 1) * F], reg, COPY, scale=dcol[:, k : k + 1]
        )

    # ---- main matmul, transposed output: 8 banks = (o-chunk, r-quarter) ----
    banks = {}
    for oc in range(FT):
        for rc in range(RQ):
            if oc == FT - 1 and rc == RQ - 1:
                banks[(oc, rc)] = psum.tile(
                    [P, 2 * F], F32, tag="mm1", bufs=1, name="bank_mm1"
                )
            else:
                banks[(oc, rc)] = psum.tile(
                    [P, 2 * F], F32, tag="bank", name=f"bank_{oc}_{rc}"
                )

    RW = N // RQ  # 512
    for k in range(NT):
        for oc in range(FT):
            for rc in range(RQ):
                nc.tensor.matmul(
                    banks[(oc, rc)][:, :RW],
                    y_big[:, k * F + oc * P : k * F + (oc + 1) * P],
                    at_big[:, k * N + rc * RW : k * N + (rc + 1) * RW],
                    start=(k == 0),
                    stop=(k == NT - 1),
                )

    # ---- drains (pure relu; d_r folded into ATd) + 2 output DMAs ----
    for oc in range(FT):
        for rc in range(RQ):
            src = banks[(oc, rc)][:, :RW]
            dst = out_t[:, oc * N + rc * RW : oc * N + (rc + 1) * RW]
            if rc % 2 == 0:
                nc.scalar.activation(dst, src, RELU)
            else:
                nc.vector.tensor_scalar(
                    out=dst, in0=src, scalar1=0.0, scalar2=None, op0=MAX
                )
        nc.sync.dma_start(
            out=OUT[oc * P : (oc + 1) * P, :], in_=out_t[:, oc * N : (oc + 1) * N]
        )


_cached_nc = None


def _build():
    nc = bacc.Bacc("TRN2", target_bir_lowering=False, debug=False)
    AT = nc.dram_tensor("at", [N, N], BF16, kind="ExternalInput").ap()
    XT = nc.dram_tensor("xt", [F, N], BF16, kind="ExternalInput").ap()
    WT = nc.dram_tensor("wt", [F, F], BF16, kind="ExternalInput").ap()
    BIAS = nc.dram_tensor("bias", [1, F], BF16, kind="ExternalInput").ap()
    DCOL = nc.dram_tensor("dcol", [P, NT], F32, kind="ExternalInput").ap()
    OUT = nc.dram_tensor("out", [F, N], BF16, kind="ExternalOutput").ap()
    with tile.TileContext(nc) as tc:
        with ExitStack() as ctx:
            _emit(ctx, tc, AT, XT, WT, BIAS, DCOL, OUT)
    nc.compile()
    return nc


def get_nc():
    global _cached_nc
    if _cached_nc is None:
        _cached_nc = _build()
    return _cached_nc


def make_in_maps(node_features, adj_matrix, W, b):
    node_features = np.asarray(node_features, dtype=np.float32)
    adj_matrix = np.asarray(adj_matrix, dtype=np.float32)
    wt = np.ascontiguousarray(np.asarray(W, dtype=np.float32).T.astype(BF))
    bias = np.ascontiguousarray(
        np.asarray(b, dtype=np.float32).astype(BF).reshape(1, F)
    )
    maps = []
    for c in range(B):
        adj = adj_matrix[c]
        deg = adj.sum(axis=1, dtype=np.float32)
        with np.errstate(divide="ignore"):
            d = deg**-0.5
        d = np.where(np.isfinite(d), d, 0.0).astype(np.float32)
        maps.append(
            {
                # (D^-1/2 A)^T: row scale folded in before the bf16 cast
                "at": np.ascontiguousarray((adj * d[:, None]).astype(BF).T),
                "xt": np.ascontiguousarray(node_features[c].T.astype(BF)),
                "wt": wt,
                "bias": bias,
                "dcol": np.ascontiguousarray(d.reshape(NT, P).T),
            }
        )
    return maps


def unpack_out(arr):
    """Device output [F, N] bf16 -> full-precision [N, F] f32."""
    return np.ascontiguousarray(np.asarray(arr).astype(np.float32).T)


def kernel(node_features, adj_matrix, W, b):
    nc = get_nc()
    in_maps = make_in_maps(node_features, adj_matrix, W, b)
    res = run_bass_kernel_spmd(nc, in_maps, core_ids=list(range(B)))
    return np.stack([unpack_out(r["out"]) for r in res.results], axis=0)
